# revision 16
# baseline (speedup 1.0000x reference)
"""Trainium2 Bass kernel for a diffusers-style cross-attention block.

Problem (hardcoded shapes):
    hidden_states         [2, 2048, 1280] f32
    encoder_hidden_states [2, 2048, 1024] f32
    Wq [1280, 1280]  Wk/Wv [1024, 1280]  Wo [1280, 1280]  b_o [1280]  (all f32)
    out = softmax((x Wq) (enc Wk)^T / 8) (enc Wv) Wo + b_o      (20 heads x 64)

Sharding across 8 NeuronCores: data-parallel on batch (2) x tensor-parallel on
heads (4 groups of 5 heads). Each core computes a partial output
[2048, 1280] = A_local @ Wo_rows for its 5 heads; the host sums the 4 partials
per batch element and adds the bias.

Per-core layout trick: the host passes TRANSPOSED activations (x^T, enc^T,
bf16), so Q^T and K^T come straight out of the projection matmuls, scores are
computed as S^T (kv on partitions, q on free), exp runs on the scalar engine
PSUM->SBUF, and the PV matmul consumes P^T directly with V stored naturally
[kv, d]. A ones-column appended to V makes the PV matmul also emit the softmax
denominator l[q]. No on-chip transposes anywhere.

Head pairs are packed into the 128 partitions (rows 0-63 / 64-127) and their
score matmuls are emitted back-to-back: lhsT base partitions 0/64 lower to PE
tile_position (0,0)/(64,0), so the two K=64 matmuls run concurrently in
disjoint row-groups of the systolic array.
"""

import numpy as np
import ml_dtypes
from contextlib import ExitStack

S = 2048          # seq len (q and kv)
C = 1280          # hidden
CC = 1024         # encoder hidden
HG = 5            # heads per core
D = 64            # head dim
HD = HG * D       # 320
VW = D + 1        # V columns incl. ones column
CK = C // 128     # 10
CCK = CC // 128   # 8
NKV = S // 128    # 16
NQ = S // 512     # 4

_CACHED = {}


def _emit(ctx, tc, xT, encT, wq, wk, wv, wo, out):
    from concourse import mybir

    nc = tc.nc
    bf16, f32 = mybir.dt.bfloat16, mybir.dt.float32
    Exp = mybir.ActivationFunctionType.Exp

    const = ctx.enter_context(tc.tile_pool(name="const", bufs=1))
    acts = ctx.enter_context(tc.tile_pool(name="acts", bufs=1))
    small = ctx.enter_context(tc.tile_pool(name="small", bufs=2))
    osb_pool = ctx.enter_context(tc.tile_pool(name="osb", bufs=2))
    phat_pool = ctx.enter_context(tc.tile_pool(name="phat", bufs=20))
    psum = ctx.enter_context(tc.tile_pool(name="psum", bufs=2, space="PSUM"))

    # ---- DMA in, critical-path order: wk, enc^T, wq, x^T, wv, wo ----
    wk_sb = const.tile([128, CCK * HD], bf16, tag="wk")
    nc.sync.dma_start(wk_sb[:], wk.rearrange("(k p) d -> p k d", p=128))
    encT_big = acts.tile([128, CCK * S], bf16, tag="encT")
    for c0, c1 in ((0, 4), (4, 8)):
        nc.sync.dma_start(
            encT_big[:, c0 * S:c1 * S],
            encT[c0 * 128:c1 * 128, :].rearrange("(k p) s -> p k s", p=128),
        )
    encT_sb = [encT_big[:, k * S:(k + 1) * S] for k in range(CCK)]
    wq_sb = const.tile([128, CK * HD], bf16, tag="wq")
    nc.sync.dma_start(wq_sb[:], wq.rearrange("(k p) d -> p k d", p=128))
    xT_big = acts.tile([128, CK * S], bf16, tag="xT")
    for c0, c1 in ((0, 4), (4, 8), (8, 10)):
        nc.sync.dma_start(
            xT_big[:, c0 * S:c1 * S],
            xT[c0 * 128:c1 * 128, :].rearrange("(k p) s -> p k s", p=128),
        )
    xT_sb = [xT_big[:, k * S:(k + 1) * S] for k in range(CK)]
    wv_sb = const.tile([128, CCK * HD], bf16, tag="wv")
    nc.sync.dma_start(wv_sb[:], wv.rearrange("(k p) d -> p k d", p=128))
    wo_sb = []
    for t in range(3):
        K = 128 if t < 2 else 64
        w = const.tile([128, C], bf16, tag=f"wo{t}", name=f"wo{t}")
        nc.sync.dma_start(w[:K, :], wo[t * 128:t * 128 + K, :])
        wo_sb.append(w)

    # persistent intermediates (head pairs packed into 128 partitions)
    qt_sb = [acts.tile([128, S], bf16, tag=f"qt{t}", name=f"qt{t}") for t in range(3)]
    kt_sb = [acts.tile([128, S], bf16, tag=f"kt{t}", name=f"kt{t}") for t in range(3)]
    at_sb = [acts.tile([128, S], bf16, tag=f"at{t}", name=f"at{t}") for t in range(3)]
    v_sb = acts.tile([128, NKV * HG * VW], bf16, tag="v")
    nc.vector.memset(v_sb[:], 1.0)  # ones columns; V blocks overwritten below

    def proj_qk_steps(w_sb, src_sb, nk, dst, t, group):
        """Generator: emits the Q/K projection for tile t in ~group-MM slices."""
        M = 128 if t < 2 else 64
        for j in range(NQ):
            ps = psum.tile([128, 512], f32, tag="s", name="ps", bufs=4)
            for k in range(nk):
                nc.tensor.matmul(
                    ps[:M, :],
                    lhsT=w_sb[:, k * HD + t * 128: k * HD + t * 128 + M],
                    rhs=src_sb[k][:, j * 512:(j + 1) * 512],
                    start=(k == 0), stop=(k == nk - 1),
                )
                if (k + 1) % group == 0:
                    yield
            nc.vector.tensor_copy(dst[:M, j * 512:(j + 1) * 512], ps[:M, :])
            yield

    def proj_qk(w_sb, src_sb, nk, dst, t):
        for _ in proj_qk_steps(w_sb, src_sb, nk, dst, t, group=999):
            pass

    def proj_v_tile(i):
        # one kv-tile of the V projection (+ ones column layout in v_sb)
        ps = psum.tile([128, 512], f32, tag="s", name="ps", bufs=4)
        for k in range(CCK):
            nc.tensor.matmul(
                ps[:, :HD],
                lhsT=encT_sb[k][:, i * 128:(i + 1) * 128],
                rhs=wv_sb[:, k * HD:(k + 1) * HD],
                start=(k == 0), stop=(k == CCK - 1),
            )
        for h in range(HG):
            nc.vector.tensor_copy(
                v_sb[:, (i * HG + h) * VW: (i * HG + h) * VW + D],
                ps[:, h * D:(h + 1) * D],
            )

    def outproj_steps(ms, group):
        for m in ms:
            osb = osb_pool.tile([128, C], f32, tag="osb", name="osb")
            cnt = 0
            for c0 in range(0, C, 512):
                cn = min(512, C - c0)
                ps = psum.tile([128, 512], f32, tag="s", name="ops", bufs=4)
                for t in range(3):
                    K = 128 if t < 2 else 64
                    nc.tensor.matmul(
                        ps[:, :cn],
                        lhsT=at_sb[t][:K, m * 128:(m + 1) * 128],
                        rhs=wo_sb[t][:K, c0:c0 + cn],
                        start=(t == 0), stop=(t == 2),
                    )
                    cnt += 1
                    if cnt % group == 0:
                        yield
                nc.vector.tensor_copy(osb[:, c0:c0 + cn], ps[:, :cn])
            nc.sync.dma_start(out[m * 128:(m + 1) * 128, :], osb[:])
            yield

    def attention(t, v_interleave=False, fill=None, qsplit=((0, 2), (2, 4)),
                  fill_rate=1, fill_from=0):
        heads = (2 * t, 2 * t + 1) if t < 2 else (4,)
        for qbi, (jc0, jc1) in enumerate(qsplit):
            nch = jc1 - jc0
            drive = fill is not None and qbi >= fill_from
            w = nch * 512
            pv = {}
            for h in heads:
                pv[h] = psum.tile([128, 1024], f32, tag="pv", name="pv", bufs=2)
            for i in range(NKV):
                phs = {h: [] for h in heads}
                for jj in range(nch):
                    j = jc0 + jj
                    for h in heads:
                        rb0 = (h % 2) * 64
                        sps = psum.tile([128, 512], f32, tag="s", name="sps", bufs=4)
                        nc.tensor.matmul(
                            sps[:],
                            lhsT=kt_sb[t][rb0:rb0 + 64, i * 128:(i + 1) * 128],
                            rhs=qt_sb[t][rb0:rb0 + 64, j * 512:(j + 1) * 512],
                            start=True, stop=True,
                        )
                        ph = phat_pool.tile([128, 512], bf16, tag="ph", name="ph")
                        nc.scalar.activation(ph[:], sps[:], Exp, scale=0.125)
                        phs[h].append(ph)
                if v_interleave and jc0 == 0:
                    proj_v_tile(i)
                if drive:
                    for _ in range(fill_rate):
                        next(fill, None)
                for h in heads:
                    for jj in range(nch):
                        nc.tensor.matmul(
                            pv[h][:VW, jj * 512:(jj + 1) * 512],
                            lhsT=v_sb[:, (i * HG + h) * VW: (i * HG + h + 1) * VW],
                            rhs=phs[h][jj][:],
                            start=(i == 0), stop=(i == NKV - 1),
                        )
            for h in heads:
                rb0 = (h % 2) * 64
                r2 = small.tile([1, w], f32, tag="r2", name="r2")
                nc.vector.reciprocal(r2[:], pv[h][64:65, :w])
                rb = small.tile([64, w], f32, tag="rb", name="rb")
                nc.gpsimd.partition_broadcast(rb[:], r2[:])
                nc.vector.tensor_mul(
                    at_sb[t][rb0:rb0 + 64, jc0 * 512:jc1 * 512],
                    pv[h][0:64, :w], rb[:],
                )
            if drive:
                next(fill, None)

    def chain(*gens):
        for g in gens:
            yield from g

    fill1 = chain(proj_qk_steps(wk_sb, encT_sb, CCK, kt_sb[1], 1, 5),
                  proj_qk_steps(wq_sb, xT_sb, CK, qt_sb[1], 1, 5))
    fill2 = chain(proj_qk_steps(wk_sb, encT_sb, CCK, kt_sb[2], 2, 5),
                  proj_qk_steps(wq_sb, xT_sb, CK, qt_sb[2], 2, 5))
    fill3 = outproj_steps(range(0, 8), 3)

    proj_qk(wk_sb, encT_sb, CCK, kt_sb[0], 0)
    proj_qk(wq_sb, xT_sb, CK, qt_sb[0], 0)
    attention(0, v_interleave=True, fill=fill1)
    for _ in fill1:
        pass
    attention(1, fill=fill2)
    for _ in fill2:
        pass
    attention(2, fill=fill3, fill_rate=2, fill_from=1)
    for _ in fill3:
        pass
    for _ in outproj_steps(range(8, NKV), 999):
        pass


def build():
    if "nc" in _CACHED:
        return _CACHED["nc"]
    import concourse.tile as tile
    from concourse import bacc, mybir

    bf16, f32 = mybir.dt.bfloat16, mybir.dt.float32
    nc = bacc.Bacc("TRN2", target_bir_lowering=False, debug=False)
    xT = nc.dram_tensor("xT", [C, S], bf16, kind="ExternalInput").ap()
    encT = nc.dram_tensor("encT", [CC, S], bf16, kind="ExternalInput").ap()
    wq = nc.dram_tensor("wq", [C, HD], bf16, kind="ExternalInput").ap()
    wk = nc.dram_tensor("wk", [CC, HD], bf16, kind="ExternalInput").ap()
    wv = nc.dram_tensor("wv", [CC, HD], bf16, kind="ExternalInput").ap()
    wo = nc.dram_tensor("wo", [HD, C], bf16, kind="ExternalInput").ap()
    out = nc.dram_tensor("out", [S, C], f32, kind="ExternalOutput").ap()

    with tile.TileContext(nc) as tc:
        with ExitStack() as ctx:
            _emit(ctx, tc, xT, encT, wq, wk, wv, wo, out)
    nc.compile()
    _CACHED["nc"] = nc
    return nc


def make_in_maps(hidden_states, encoder_hidden_states, Wq, Wk, Wv, Wo):
    bf = ml_dtypes.bfloat16
    in_maps = []
    xTs = [np.ascontiguousarray(hidden_states[b].T).astype(bf) for b in range(2)]
    encTs = [np.ascontiguousarray(encoder_hidden_states[b].T).astype(bf) for b in range(2)]
    for core in range(8):
        b, g = divmod(core, 4)
        cols = slice(g * HD, (g + 1) * HD)
        in_maps.append({
            "xT": xTs[b],
            "encT": encTs[b],
            "wq": np.ascontiguousarray(Wq[:, cols]).astype(bf),
            "wk": np.ascontiguousarray(Wk[:, cols]).astype(bf),
            "wv": np.ascontiguousarray(Wv[:, cols]).astype(bf),
            "wo": np.ascontiguousarray(Wo[cols, :]).astype(bf),
        })
    return in_maps


def kernel(hidden_states, encoder_hidden_states, Wq, Wk, Wv, Wo, b_o):
    from concourse.bass_utils import run_bass_kernel_spmd

    nc = build()
    in_maps = make_in_maps(hidden_states, encoder_hidden_states, Wq, Wk, Wv, Wo)
    res = run_bass_kernel_spmd(nc, in_maps, core_ids=list(range(8)))
    outs = [res.results[c]["out"] for c in range(8)]
    full = np.stack([
        outs[0] + outs[1] + outs[2] + outs[3],
        outs[4] + outs[5] + outs[6] + outs[7],
    ]).astype(np.float32)
    full += np.asarray(b_o, np.float32)
    return full


# revision 27
# speedup vs baseline: 1.0679x; 1.0679x over previous
"""Trainium2 Bass kernel for a diffusers-style cross-attention block.

Problem (hardcoded shapes):
    hidden_states         [2, 2048, 1280] f32
    encoder_hidden_states [2, 2048, 1024] f32
    Wq [1280, 1280]  Wk/Wv [1024, 1280]  Wo [1280, 1280]  b_o [1280]  (all f32)
    out = softmax((x Wq) (enc Wk)^T / 8) (enc Wv) Wo + b_o      (20 heads x 64)

Sharding across 8 NeuronCores: data-parallel on batch (2) x tensor-parallel on
heads (4 groups of 5 heads). Each core computes a partial output
[2048, 1280] = A_local @ Wo_rows for its 5 heads; the host sums the 4 partials
per batch element and adds the bias.

Per-core layout trick: the host passes TRANSPOSED activations (x^T, enc^T,
bf16), so Q^T and K^T come straight out of the projection matmuls, scores are
computed as S^T (kv on partitions, q on free), exp runs on the scalar engine
PSUM->SBUF, and the PV matmul consumes P^T directly with V stored naturally
[kv, d]. A ones-column appended to V makes the PV matmul also emit the softmax
denominator l[q]. No on-chip transposes anywhere.

Head pairs are packed into the 128 partitions (rows 0-63 / 64-127) and their
score matmuls are emitted back-to-back: lhsT base partitions 0/64 lower to PE
tile_position (0,0)/(64,0), so the two K=64 matmuls run concurrently in
disjoint row-groups of the systolic array.
"""

import numpy as np
import ml_dtypes
from contextlib import ExitStack

S = 2048          # seq len (q and kv)
C = 1280          # hidden
CC = 1024         # encoder hidden
HG = 5            # heads per core
D = 64            # head dim
HD = HG * D       # 320
VW = D + 1        # V columns incl. ones column
CK = C // 128     # 10
CCK = CC // 128   # 8
NKV = S // 128    # 16
NQ = S // 512     # 4

_CACHED = {}


def _emit(ctx, tc, xT, encT, wq, wk, wv, wo, out):
    from concourse import mybir

    nc = tc.nc
    bf16, f32 = mybir.dt.bfloat16, mybir.dt.float32
    Exp = mybir.ActivationFunctionType.Exp

    const = ctx.enter_context(tc.tile_pool(name="const", bufs=1))
    acts = ctx.enter_context(tc.tile_pool(name="acts", bufs=1))
    small = ctx.enter_context(tc.tile_pool(name="small", bufs=2))
    osb_pool = ctx.enter_context(tc.tile_pool(name="osb", bufs=3))
    phat_pool = ctx.enter_context(tc.tile_pool(name="phat", bufs=24))
    psum = ctx.enter_context(tc.tile_pool(name="psum", bufs=2, space="PSUM"))

    # ---- DMA in, critical-path order: wk, enc^T, wq, x^T, wv, wo ----
    wk_sb = const.tile([128, CCK * HD], bf16, tag="wk")
    nc.sync.dma_start(wk_sb[:], wk.rearrange("(k p) d -> p k d", p=128))
    encT_big = acts.tile([128, CCK * S], bf16, tag="encT")
    for c0, c1 in ((0, 4), (4, 8)):
        nc.sync.dma_start(
            encT_big[:, c0 * S:c1 * S],
            encT[c0 * 128:c1 * 128, :].rearrange("(k p) s -> p k s", p=128),
        )
    encT_sb = [encT_big[:, k * S:(k + 1) * S] for k in range(CCK)]
    wq_sb = const.tile([128, CK * HD], bf16, tag="wq")
    nc.sync.dma_start(wq_sb[:], wq.rearrange("(k p) d -> p k d", p=128))
    xT_big = acts.tile([128, CK * S], bf16, tag="xT")
    for c0, c1 in ((0, 4), (4, 8), (8, 10)):
        nc.sync.dma_start(
            xT_big[:, c0 * S:c1 * S],
            xT[c0 * 128:c1 * 128, :].rearrange("(k p) s -> p k s", p=128),
        )
    xT_sb = [xT_big[:, k * S:(k + 1) * S] for k in range(CK)]
    wv_sb = const.tile([128, CCK * HD], bf16, tag="wv")
    nc.sync.dma_start(wv_sb[:], wv.rearrange("(k p) d -> p k d", p=128))
    wo_sb = []
    for t in range(3):
        K = 128 if t < 2 else 64
        w = const.tile([128, C], bf16, tag=f"wo{t}", name=f"wo{t}")
        nc.sync.dma_start(w[:K, :], wo[t * 128:t * 128 + K, :])
        wo_sb.append(w)

    # persistent intermediates (head pairs packed into 128 partitions)
    qt_sb = [acts.tile([128, S], bf16, tag=f"qt{t}", name=f"qt{t}") for t in range(3)]
    kt_sb = [acts.tile([128, S], bf16, tag=f"kt{t}", name=f"kt{t}") for t in range(3)]
    at_sb = [acts.tile([128, S], bf16, tag=f"at{t}", name=f"at{t}") for t in range(3)]
    v_sb = acts.tile([128, NKV * HG * VW], bf16, tag="v")
    nc.vector.memset(v_sb[:], 1.0)  # ones columns; V blocks overwritten below

    def proj_qk_steps(w_sb, src_sb, nk, dst, t, group):
        """Generator: emits the Q/K projection for tile t in ~group-MM slices."""
        M = 128 if t < 2 else 64
        for j in range(NQ):
            ps = psum.tile([128, 512], f32, tag="s", name="ps", bufs=4)
            for k in range(nk):
                nc.tensor.matmul(
                    ps[:M, :],
                    lhsT=w_sb[:, k * HD + t * 128: k * HD + t * 128 + M],
                    rhs=src_sb[k][:, j * 512:(j + 1) * 512],
                    start=(k == 0), stop=(k == nk - 1),
                )
                if (k + 1) % group == 0:
                    yield
            nc.vector.tensor_copy(dst[:M, j * 512:(j + 1) * 512], ps[:M, :])
            yield

    def proj_qk(w_sb, src_sb, nk, dst, t):
        for _ in proj_qk_steps(w_sb, src_sb, nk, dst, t, group=999):
            pass

    def proj_v_tile(i):
        # one kv-tile of the V projection (+ ones column layout in v_sb)
        ps = psum.tile([128, 512], f32, tag="fill", name="ps", bufs=2)
        for k in range(CCK):
            nc.tensor.matmul(
                ps[:, :HD],
                lhsT=encT_sb[k][:, i * 128:(i + 1) * 128],
                rhs=wv_sb[:, k * HD:(k + 1) * HD],
                start=(k == 0), stop=(k == CCK - 1),
            )
        for h in range(HG):
            nc.vector.tensor_copy(
                v_sb[:, (i * HG + h) * VW: (i * HG + h) * VW + D],
                ps[:, h * D:(h + 1) * D],
            )

    def proj_qk_fill_steps(w_sb, src_sb, nk, dst, t, group):
        M = 128 if t < 2 else 64
        for j in range(NQ):
            ps = psum.tile([128, 512], f32, tag="fill", name="ps", bufs=2)
            for k in range(nk):
                nc.tensor.matmul(
                    ps[:M, :],
                    lhsT=w_sb[:, k * HD + t * 128: k * HD + t * 128 + M],
                    rhs=src_sb[k][:, j * 512:(j + 1) * 512],
                    start=(k == 0), stop=(k == nk - 1),
                )
                if (k + 1) % group == 0:
                    yield
            nc.vector.tensor_copy(dst[:M, j * 512:(j + 1) * 512], ps[:M, :])
            yield

    def outproj_steps(ms, group):
        for m in ms:
            osb = osb_pool.tile([128, C], f32, tag="osb", name="osb")
            cnt = 0
            for c0 in range(0, C, 512):
                cn = min(512, C - c0)
                ps = psum.tile([128, 512], f32, tag="fill", name="ops", bufs=2)
                for t in range(3):
                    K = 128 if t < 2 else 64
                    nc.tensor.matmul(
                        ps[:, :cn],
                        lhsT=at_sb[t][:K, m * 128:(m + 1) * 128],
                        rhs=wo_sb[t][:K, c0:c0 + cn],
                        start=(t == 0), stop=(t == 2),
                    )
                    cnt += 1
                    if cnt % group == 0:
                        yield
                nc.vector.tensor_copy(osb[:, c0:c0 + cn], ps[:, :cn])
            nc.sync.dma_start(out[m * 128:(m + 1) * 128, :], osb[:])
            yield

    def attention(t, v_interleave=False, fills=(None, None, None, None),
                  fill_rate=2):
        """Four 512-wide q-block rounds; fills[r] is a generator driven during
        round r (must only read data produced in rounds < r)."""
        heads = (2 * t, 2 * t + 1) if t < 2 else (4,)
        for jb in range(NQ):
            fill = fills[jb]
            pv = {}
            for h in heads:
                pv[h] = psum.tile([128, 512], f32, tag="pv", name="pv", bufs=2)
            for i in range(NKV):
                phs = {}
                for h in heads:
                    rb0 = (h % 2) * 64
                    sps = psum.tile([128, 512], f32, tag="s", name="sps", bufs=4)
                    nc.tensor.matmul(
                        sps[:],
                        lhsT=kt_sb[t][rb0:rb0 + 64, i * 128:(i + 1) * 128],
                        rhs=qt_sb[t][rb0:rb0 + 64, jb * 512:(jb + 1) * 512],
                        start=True, stop=True,
                    )
                    ph = phat_pool.tile([128, 512], bf16, tag="ph", name="ph")
                    nc.scalar.activation(ph[:], sps[:], Exp, scale=0.125)
                    phs[h] = ph
                if v_interleave and jb == 0:
                    proj_v_tile(i)
                if fill is not None:
                    for _ in range(fill_rate):
                        next(fill, None)
                for h in heads:
                    nc.tensor.matmul(
                        pv[h][:VW, :],
                        lhsT=v_sb[:, (i * HG + h) * VW: (i * HG + h + 1) * VW],
                        rhs=phs[h][:],
                        start=(i == 0), stop=(i == NKV - 1),
                    )
            for h in heads:
                rb0 = (h % 2) * 64
                r2 = small.tile([1, 512], f32, tag="r2", name="r2")
                nc.vector.reciprocal(r2[:], pv[h][64:65, :])
                rb = small.tile([64, 512], f32, tag="rb", name="rb")
                nc.gpsimd.partition_broadcast(rb[:], r2[:])
                nc.vector.tensor_mul(
                    at_sb[t][rb0:rb0 + 64, jb * 512:(jb + 1) * 512],
                    pv[h][0:64, :], rb[:],
                )
            if fill is not None:
                next(fill, None)

    def chain(*gens):
        for g in gens:
            yield from g

    def drain(g):
        for _ in g:
            pass

    fill1 = chain(proj_qk_fill_steps(wk_sb, encT_sb, CCK, kt_sb[1], 1, 4),
                  proj_qk_fill_steps(wq_sb, xT_sb, CK, qt_sb[1], 1, 4))
    fill2 = chain(proj_qk_fill_steps(wk_sb, encT_sb, CCK, kt_sb[2], 2, 4),
                  proj_qk_fill_steps(wq_sb, xT_sb, CK, qt_sb[2], 2, 4))

    proj_qk(wk_sb, encT_sb, CCK, kt_sb[0], 0)
    proj_qk(wq_sb, xT_sb, CK, qt_sb[0], 0)
    attention(0, v_interleave=True, fills=(None, fill1, fill1, fill1))
    drain(fill1)
    attention(1, fills=(fill2, fill2, fill2, fill2))
    drain(fill2)
    # out-projection row-group k (m = 4k..4k+3) reads at columns produced by
    # round k, so it may only be driven in rounds > k.
    og = [outproj_steps(range(4 * k, 4 * k + 4), 2) for k in range(3)]
    attention(2, fills=(None, og[0], og[1], og[2]), fill_rate=3)
    for g in og:
        drain(g)
    drain(outproj_steps(range(12, NKV), 999))


def build():
    if "nc" in _CACHED:
        return _CACHED["nc"]
    import concourse.tile as tile
    from concourse import bacc, mybir

    bf16, f32 = mybir.dt.bfloat16, mybir.dt.float32
    nc = bacc.Bacc("TRN2", target_bir_lowering=False, debug=False)
    xT = nc.dram_tensor("xT", [C, S], bf16, kind="ExternalInput").ap()
    encT = nc.dram_tensor("encT", [CC, S], bf16, kind="ExternalInput").ap()
    wq = nc.dram_tensor("wq", [C, HD], bf16, kind="ExternalInput").ap()
    wk = nc.dram_tensor("wk", [CC, HD], bf16, kind="ExternalInput").ap()
    wv = nc.dram_tensor("wv", [CC, HD], bf16, kind="ExternalInput").ap()
    wo = nc.dram_tensor("wo", [HD, C], bf16, kind="ExternalInput").ap()
    out = nc.dram_tensor("out", [S, C], f32, kind="ExternalOutput").ap()

    with tile.TileContext(nc) as tc:
        with ExitStack() as ctx:
            _emit(ctx, tc, xT, encT, wq, wk, wv, wo, out)
    nc.compile()
    _CACHED["nc"] = nc
    return nc


def make_in_maps(hidden_states, encoder_hidden_states, Wq, Wk, Wv, Wo):
    bf = ml_dtypes.bfloat16
    in_maps = []
    xTs = [np.ascontiguousarray(hidden_states[b].T).astype(bf) for b in range(2)]
    encTs = [np.ascontiguousarray(encoder_hidden_states[b].T).astype(bf) for b in range(2)]
    for core in range(8):
        b, g = divmod(core, 4)
        cols = slice(g * HD, (g + 1) * HD)
        in_maps.append({
            "xT": xTs[b],
            "encT": encTs[b],
            "wq": np.ascontiguousarray(Wq[:, cols]).astype(bf),
            "wk": np.ascontiguousarray(Wk[:, cols]).astype(bf),
            "wv": np.ascontiguousarray(Wv[:, cols]).astype(bf),
            "wo": np.ascontiguousarray(Wo[cols, :]).astype(bf),
        })
    return in_maps


def kernel(hidden_states, encoder_hidden_states, Wq, Wk, Wv, Wo, b_o):
    from concourse.bass_utils import run_bass_kernel_spmd

    nc = build()
    in_maps = make_in_maps(hidden_states, encoder_hidden_states, Wq, Wk, Wv, Wo)
    res = run_bass_kernel_spmd(nc, in_maps, core_ids=list(range(8)))
    outs = [res.results[c]["out"] for c in range(8)]
    full = np.stack([
        outs[0] + outs[1] + outs[2] + outs[3],
        outs[4] + outs[5] + outs[6] + outs[7],
    ]).astype(np.float32)
    full += np.asarray(b_o, np.float32)
    return full
